# revision 10
# baseline (speedup 1.0000x reference)
"""Fused FP8-block-quantized MLP (silu(x@w1.T) * (x@w3.T)) @ w2.T on 8 trn2 cores.

Sharding: data-parallel over tokens. Each core gets T/8 = 512 tokens and the
full (dequantized, bf16) weights; there are no collectives. Host-side prep
dequantizes the block-quantized weights, casts to bf16, and lays tensors out
partition-major so every device DMA is contiguous.

Device kernel per core (all matmuls bf16, fp32 PSUM accumulation):
  phase A: for each 128-row block fb of F: g.T/u.T [128f, 512t] accumulated
           over 16 k-blocks of H; silu on ACT, mul on DVE -> fusedT in SBUF.
  phase B: out [512t, 2048h] = fusedT.T @ w2.T, streaming w2 column blocks,
           accumulating over the 56 f-blocks in PSUM.
"""

import sys

import numpy as np

_REPO = "/opt/trn_rl_repo"
if _REPO not in sys.path:
    sys.path.insert(0, _REPO)

T, H, F = 4096, 2048, 7168
NCORES = 8
TC = T // NCORES      # 512 tokens per core
KB = H // 128         # 16 contraction blocks for matmul 1/3
FB = F // 128         # 56 f blocks
HCOLS = H // 512      # 4 output column groups
TB = TC // 128        # 4 token blocks

_CACHE = {}


def _build_program():
    import concourse.mybir as mybir
    from concourse import bacc
    from concourse.tile import TileContext

    bf16 = mybir.dt.bfloat16
    f32 = mybir.dt.float32

    # Bacc (not bass.Bass): its finalize() runs generate_event_semaphores,
    # which splits multi-wait sync_info into EventSemaphore instructions —
    # TRN2 instructions physically carry at most one sem wait.
    nc = bacc.Bacc()
    xt_d = nc.declare_dram_parameter("xt", [KB, 128, TC], bf16, isOutput=False)
    w1_d = nc.declare_dram_parameter("w1p", [FB, 128, H], bf16, isOutput=False)
    w3_d = nc.declare_dram_parameter("w3p", [FB, 128, H], bf16, isOutput=False)
    w2_d = nc.declare_dram_parameter(
        "w2p", [HCOLS, FB, 128, 512], bf16, isOutput=False
    )
    out_d = nc.declare_dram_parameter("out", [TC, H], f32, isOutput=True)

    with TileContext(nc) as tc:
        with (
            # HWDGE DMA lanes are assigned round-robin by emission order;
            # slot-reuse distance must be a multiple of 8 DMAs so the WAW
            # dep stays on the same lane (DMA instrs max out at 2 waits).
            tc.tile_pool(name="xpool", bufs=1) as xpool,
            tc.tile_pool(name="w1pool", bufs=4) as w1pool,
            tc.tile_pool(name="w3pool", bufs=4) as w3pool,
            tc.tile_pool(name="w2pool", bufs=8) as w2pool,
            tc.tile_pool(name="sgpool", bufs=3) as sgpool,
            tc.tile_pool(name="upool", bufs=3) as upool,
            tc.tile_pool(name="fpool", bufs=FB) as fpool,
            tc.tile_pool(name="opool", bufs=HCOLS * TB) as opool,
            tc.tile_pool(name="psg", bufs=2, space="PSUM") as psg,
            tc.tile_pool(name="psu", bufs=2, space="PSUM") as psu,
            tc.tile_pool(name="psb", bufs=4, space="PSUM") as psb,
        ):
            xtile = xpool.tile([128, KB, TC], bf16)

            fused = []
            for fb in range(FB):
                w1t = w1pool.tile([128, H], bf16, tag="w1t")
                w3t = w3pool.tile([128, H], bf16, tag="w3t")
                gps = psg.tile([128, TC], f32, tag="gps")
                ups = psu.tile([128, TC], f32, tag="ups")
                if fb == 0:
                    # Startup: interleave the x-transpose loads with
                    # per-k-block chunks of the first weight block, and
                    # alternate g/u matmuls per k-block — the first matmul
                    # then waits on ~200KB of DMA instead of ~3MB.
                    for kb in range(KB):
                        sl = slice(kb * 128, (kb + 1) * 128)
                        nc.sync.dma_start(
                            out=xtile[:, kb, :], in_=xt_d[kb]
                        )
                        nc.sync.dma_start(out=w1t[:, sl], in_=w1_d[fb][:, sl])
                        nc.sync.dma_start(out=w3t[:, sl], in_=w3_d[fb][:, sl])
                    for kb in range(KB):
                        sl = slice(kb * 128, (kb + 1) * 128)
                        nc.tensor.matmul(
                            gps,
                            w1t[:, sl],
                            xtile[:, kb, :],
                            start=(kb == 0),
                            stop=(kb == KB - 1),
                        )
                        nc.tensor.matmul(
                            ups,
                            w3t[:, sl],
                            xtile[:, kb, :],
                            start=(kb == 0),
                            stop=(kb == KB - 1),
                        )
                else:
                    nc.sync.dma_start(out=w1t, in_=w1_d[fb])
                    nc.sync.dma_start(out=w3t, in_=w3_d[fb])
                    for kb in range(KB):
                        nc.tensor.matmul(
                            gps,
                            w1t[:, kb * 128 : (kb + 1) * 128],
                            xtile[:, kb, :],
                            start=(kb == 0),
                            stop=(kb == KB - 1),
                        )
                    for kb in range(KB):
                        nc.tensor.matmul(
                            ups,
                            w3t[:, kb * 128 : (kb + 1) * 128],
                            xtile[:, kb, :],
                            start=(kb == 0),
                            stop=(kb == KB - 1),
                        )

                # Both TT inputs must come from one engine: TensorTensor
                # instructions have a single sync-wait slot. ACT evacuates
                # both PSUM banks (Silu and Copy share one ACT table, so
                # alternating them reloads nothing); the DVE multiply then
                # waits on ACT alone.
                sg = sgpool.tile([128, TC], f32, tag="sg")
                nc.scalar.activation(
                    sg, gps, mybir.ActivationFunctionType.Silu
                )
                usb = upool.tile([128, TC], f32, tag="usb")
                nc.scalar.copy(usb, ups)
                fut = fpool.tile(
                    [128, TC], bf16, tag="fused", name=f"fused{fb}"
                )
                nc.vector.tensor_tensor(
                    fut, sg, usb, mybir.AluOpType.mult
                )
                fused.append(fut)

            outs = []
            for hc in range(HCOLS):
                pss = []
                for tb in range(TB):
                    ps = psb.tile(
                        [128, 512], f32, tag="pss", name=f"pss{hc}_{tb}"
                    )
                    pss.append(ps)
                for fb in range(FB):
                    w2t = w2pool.tile([128, 512], bf16, tag="w2t")
                    nc.sync.dma_start(out=w2t, in_=w2_d[hc, fb])
                    for tb in range(TB):
                        nc.tensor.matmul(
                            pss[tb],
                            fused[fb][:, tb * 128 : (tb + 1) * 128],
                            w2t,
                            start=(fb == 0),
                            stop=(fb == FB - 1),
                        )
                for tb in range(TB):
                    ot = opool.tile(
                        [128, 512], f32, tag="ot", name=f"ot{hc}_{tb}"
                    )
                    nc.vector.tensor_copy(ot, pss[tb])
                    outs.append((hc, tb, ot))
            # Output stores all at the end: keeps the 56-per-hc w2 DMA
            # cadence aligned to the 8 HWDGE lanes, and these never reuse
            # slots so they carry at most one wait.
            for hc, tb, ot in outs:
                nc.sync.dma_start(
                    out=out_d[
                        tb * 128 : (tb + 1) * 128,
                        hc * 512 : (hc + 1) * 512,
                    ],
                    in_=ot,
                )
    nc.finalize()
    return nc


def _dequant(wq, s):
    wq = np.asarray(wq, dtype=np.float32)
    s = np.asarray(s, dtype=np.float32)
    n, k = wq.shape
    nb, kb = s.shape
    w = wq.reshape(nb, n // nb, kb, k // kb) * s[:, None, :, None]
    return w.reshape(n, k)


def _prep_inputs(hidden_states, w1_q, w1_s, w3_q, w3_s, w2_q, w2_s):
    import ml_dtypes

    bf = ml_dtypes.bfloat16

    w1 = _dequant(w1_q, w1_s).astype(bf)  # [F, H]
    w3 = _dequant(w3_q, w3_s).astype(bf)  # [F, H]
    w2 = _dequant(w2_q, w2_s).astype(bf)  # [H, F]

    # w1p[fb, p, kb*128+c] = w1[fb*128+c, kb*128+p]
    w1p = np.ascontiguousarray(
        w1.reshape(FB, 128, KB, 128).transpose(0, 3, 2, 1)
    ).reshape(FB, 128, H)
    w3p = np.ascontiguousarray(
        w3.reshape(FB, 128, KB, 128).transpose(0, 3, 2, 1)
    ).reshape(FB, 128, H)
    # w2p[hc, fb, p, c] = w2[hc*512+c, fb*128+p]
    w2p = np.ascontiguousarray(
        np.asarray(w2).reshape(HCOLS, 512, FB, 128).transpose(0, 2, 3, 1)
    )

    x = np.asarray(hidden_states, dtype=np.float32).astype(bf)
    xts = []
    for c in range(NCORES):
        xc = x[c * TC : (c + 1) * TC, :]
        # xt[kb, p, t] = xc[t, kb*128+p]
        xts.append(
            np.ascontiguousarray(xc.reshape(TC, KB, 128).transpose(1, 2, 0))
        )

    return [
        {"xt": xts[c], "w1p": w1p, "w3p": w3p, "w2p": w2p}
        for c in range(NCORES)
    ]


def _run(in_maps, **kwargs):
    from concourse.bass_utils import run_bass_kernel_spmd

    if "nc" not in _CACHE:
        _CACHE["nc"] = _build_program()
    res = run_bass_kernel_spmd(
        _CACHE["nc"], in_maps, list(range(NCORES)), **kwargs
    )
    out = np.concatenate(
        [res.results[c]["out"] for c in range(NCORES)], axis=0
    )
    return np.asarray(out, dtype=np.float32), res


def kernel(hidden_states, w1_q, w1_s, w3_q, w3_s, w2_q, w2_s):
    in_maps = _prep_inputs(
        hidden_states, w1_q, w1_s, w3_q, w3_s, w2_q, w2_s
    )
    out, _ = _run(in_maps)
    return out


# revision 12
# speedup vs baseline: 1.0218x; 1.0218x over previous
"""Fused FP8-block-quantized MLP (silu(x@w1.T) * (x@w3.T)) @ w2.T on 8 trn2 cores.

Sharding: data-parallel over tokens. Each core gets T/8 = 512 tokens and the
full (dequantized, bf16) weights; there are no collectives. Host-side prep
dequantizes the block-quantized weights, casts to bf16, and lays tensors out
partition-major so every device DMA is contiguous.

Device kernel per core (all matmuls bf16, fp32 PSUM accumulation):
  phase A: for each 128-row block fb of F: g.T/u.T [128f, 512t] accumulated
           over 16 k-blocks of H; silu on ACT, mul on DVE -> fusedT in SBUF.
  phase B: out [512t, 2048h] = fusedT.T @ w2.T, streaming w2 column blocks,
           accumulating over the 56 f-blocks in PSUM.
"""

import sys

import numpy as np

_REPO = "/opt/trn_rl_repo"
if _REPO not in sys.path:
    sys.path.insert(0, _REPO)

T, H, F = 4096, 2048, 7168
NCORES = 8
TC = T // NCORES      # 512 tokens per core
KB = H // 128         # 16 contraction blocks for matmul 1/3
FB = F // 128         # 56 f blocks
HCOLS = H // 512      # 4 output column groups
TB = TC // 128        # 4 token blocks

_CACHE = {}


def _build_program():
    import concourse.mybir as mybir
    from concourse import bacc
    from concourse.tile import TileContext

    bf16 = mybir.dt.bfloat16
    f32 = mybir.dt.float32

    # Bacc (not bass.Bass): its finalize() runs generate_event_semaphores,
    # which splits multi-wait sync_info into EventSemaphore instructions —
    # TRN2 instructions physically carry at most one sem wait.
    nc = bacc.Bacc()
    xt_d = nc.declare_dram_parameter("xt", [KB, 128, TC], bf16, isOutput=False)
    w1_d = nc.declare_dram_parameter("w1p", [FB, 128, H], bf16, isOutput=False)
    w3_d = nc.declare_dram_parameter("w3p", [FB, 128, H], bf16, isOutput=False)
    w2_d = nc.declare_dram_parameter(
        "w2p", [HCOLS, FB, 128, 512], bf16, isOutput=False
    )
    out_d = nc.declare_dram_parameter("out", [TC, H], f32, isOutput=True)

    with TileContext(nc) as tc:
        with (
            # HWDGE DMA lanes are assigned round-robin by emission order;
            # slot-reuse distance must be a multiple of 8 DMAs so the WAW
            # dep stays on the same lane (DMA instrs max out at 2 waits).
            tc.tile_pool(name="xpool", bufs=1) as xpool,
            tc.tile_pool(name="w1pool", bufs=4) as w1pool,
            tc.tile_pool(name="w3pool", bufs=4) as w3pool,
            tc.tile_pool(name="w2pool", bufs=8) as w2pool,
            tc.tile_pool(name="sgpool", bufs=3) as sgpool,
            tc.tile_pool(name="upool", bufs=3) as upool,
            tc.tile_pool(name="fpool", bufs=FB) as fpool,
            tc.tile_pool(name="opool", bufs=HCOLS * TB) as opool,
            tc.tile_pool(name="psg", bufs=2, space="PSUM") as psg,
            tc.tile_pool(name="psu", bufs=2, space="PSUM") as psu,
            tc.tile_pool(name="psb", bufs=4, space="PSUM") as psb,
        ):
            xtile = xpool.tile([128, KB, TC], bf16)
            for kb in range(KB):
                nc.sync.dma_start(out=xtile[:, kb, :], in_=xt_d[kb])

            fused = []
            for fb in range(FB):
                w1t = w1pool.tile([128, H], bf16, tag="w1t")
                nc.sync.dma_start(out=w1t, in_=w1_d[fb])
                w3t = w3pool.tile([128, H], bf16, tag="w3t")
                nc.sync.dma_start(out=w3t, in_=w3_d[fb])

                gps = psg.tile([128, TC], f32, tag="gps")
                for kb in range(KB):
                    nc.tensor.matmul(
                        gps,
                        w1t[:, kb * 128 : (kb + 1) * 128],
                        xtile[:, kb, :],
                        start=(kb == 0),
                        stop=(kb == KB - 1),
                    )
                ups = psu.tile([128, TC], f32, tag="ups")
                for kb in range(KB):
                    nc.tensor.matmul(
                        ups,
                        w3t[:, kb * 128 : (kb + 1) * 128],
                        xtile[:, kb, :],
                        start=(kb == 0),
                        stop=(kb == KB - 1),
                    )

                # Both TT inputs must come from one engine: TensorTensor
                # instructions have a single sync-wait slot. ACT evacuates
                # both PSUM banks (Silu and Copy share one ACT table, so
                # alternating them reloads nothing); the DVE multiply then
                # waits on ACT alone.
                sg = sgpool.tile([128, TC], f32, tag="sg")
                nc.scalar.activation(
                    sg, gps, mybir.ActivationFunctionType.Silu
                )
                usb = upool.tile([128, TC], f32, tag="usb")
                nc.scalar.copy(usb, ups)
                fut = fpool.tile(
                    [128, TC], bf16, tag="fused", name=f"fused{fb}"
                )
                nc.vector.tensor_tensor(
                    fut, sg, usb, mybir.AluOpType.mult
                )
                fused.append(fut)

            for hc in range(HCOLS):
                pss = []
                for tb in range(TB):
                    ps = psb.tile(
                        [128, 512], f32, tag="pss", name=f"pss{hc}_{tb}"
                    )
                    pss.append(ps)
                for fb in range(FB):
                    w2t = w2pool.tile([128, 512], bf16, tag="w2t")
                    nc.sync.dma_start(out=w2t, in_=w2_d[hc, fb])
                    for tb in range(TB):
                        nc.tensor.matmul(
                            pss[tb],
                            fused[fb][:, tb * 128 : (tb + 1) * 128],
                            w2t,
                            start=(fb == 0),
                            stop=(fb == FB - 1),
                        )
                for tb in range(TB):
                    ot = opool.tile(
                        [128, 512], f32, tag="ot", name=f"ot{hc}_{tb}"
                    )
                    nc.vector.tensor_copy(ot, pss[tb])
                    nc.sync.dma_start(
                        out=out_d[
                            tb * 128 : (tb + 1) * 128,
                            hc * 512 : (hc + 1) * 512,
                        ],
                        in_=ot,
                    )
    nc.finalize()
    return nc


def _dequant(wq, s):
    wq = np.asarray(wq, dtype=np.float32)
    s = np.asarray(s, dtype=np.float32)
    n, k = wq.shape
    nb, kb = s.shape
    w = wq.reshape(nb, n // nb, kb, k // kb) * s[:, None, :, None]
    return w.reshape(n, k)


def _prep_inputs(hidden_states, w1_q, w1_s, w3_q, w3_s, w2_q, w2_s):
    import ml_dtypes

    bf = ml_dtypes.bfloat16

    w1 = _dequant(w1_q, w1_s).astype(bf)  # [F, H]
    w3 = _dequant(w3_q, w3_s).astype(bf)  # [F, H]
    w2 = _dequant(w2_q, w2_s).astype(bf)  # [H, F]

    # w1p[fb, p, kb*128+c] = w1[fb*128+c, kb*128+p]
    w1p = np.ascontiguousarray(
        w1.reshape(FB, 128, KB, 128).transpose(0, 3, 2, 1)
    ).reshape(FB, 128, H)
    w3p = np.ascontiguousarray(
        w3.reshape(FB, 128, KB, 128).transpose(0, 3, 2, 1)
    ).reshape(FB, 128, H)
    # w2p[hc, fb, p, c] = w2[hc*512+c, fb*128+p]
    w2p = np.ascontiguousarray(
        np.asarray(w2).reshape(HCOLS, 512, FB, 128).transpose(0, 2, 3, 1)
    )

    x = np.asarray(hidden_states, dtype=np.float32).astype(bf)
    xts = []
    for c in range(NCORES):
        xc = x[c * TC : (c + 1) * TC, :]
        # xt[kb, p, t] = xc[t, kb*128+p]
        xts.append(
            np.ascontiguousarray(xc.reshape(TC, KB, 128).transpose(1, 2, 0))
        )

    return [
        {"xt": xts[c], "w1p": w1p, "w3p": w3p, "w2p": w2p}
        for c in range(NCORES)
    ]


def _run(in_maps, **kwargs):
    from concourse.bass_utils import run_bass_kernel_spmd

    if "nc" not in _CACHE:
        _CACHE["nc"] = _build_program()
    res = run_bass_kernel_spmd(
        _CACHE["nc"], in_maps, list(range(NCORES)), **kwargs
    )
    out = np.concatenate(
        [res.results[c]["out"] for c in range(NCORES)], axis=0
    )
    return np.asarray(out, dtype=np.float32), res


def kernel(hidden_states, w1_q, w1_s, w3_q, w3_s, w2_q, w2_s):
    in_maps = _prep_inputs(
        hidden_states, w1_q, w1_s, w3_q, w3_s, w2_q, w2_s
    )
    out, _ = _run(in_maps)
    return out


# revision 14
# speedup vs baseline: 1.0300x; 1.0081x over previous
"""Fused FP8-block-quantized MLP (silu(x@w1.T) * (x@w3.T)) @ w2.T on 8 trn2 cores.

Sharding: data-parallel over tokens. Each core gets T/8 = 512 tokens and the
full (dequantized, bf16) weights; there are no collectives. Host-side prep
dequantizes the block-quantized weights, casts to bf16, and lays tensors out
partition-major so every device DMA is one large contiguous transfer.

Device kernel per core (all matmuls bf16, fp32 PSUM accumulation):
  phase A: for each 128-row block fb of F: g.T/u.T [128f, 512t] accumulated
           over 16 k-blocks of H; silu+copy on ACT, mul on DVE -> fusedT
           kept in SBUF.
  phase B: out [512t, 2048h] = fusedT.T @ w2.T, streaming w2 column blocks,
           accumulating over the 56 f-blocks in PSUM.
"""

import sys

import numpy as np

_REPO = "/opt/trn_rl_repo"
if _REPO not in sys.path:
    sys.path.insert(0, _REPO)

T, H, F = 4096, 2048, 7168
NCORES = 8
TC = T // NCORES      # 512 tokens per core
KB = H // 128         # 16 contraction blocks for matmul 1/3
FB = F // 128         # 56 f blocks
FB2 = FB // 2         # w2 blocks are streamed in pairs
HCOLS = H // 512      # 4 output column groups
TB = TC // 128        # 4 token blocks

_CACHE = {}


def _build_program():
    import concourse.mybir as mybir
    from concourse import bacc
    from concourse.tile import TileContext

    bf16 = mybir.dt.bfloat16
    f32 = mybir.dt.float32

    # Bacc (not bass.Bass): its finalize() runs generate_event_semaphores,
    # which splits multi-wait sync_info into EventSemaphore instructions —
    # TRN2 instructions physically carry at most one sem wait.
    nc = bacc.Bacc()
    # All inputs are laid out partition-major on the host so each DMA below
    # is a single large transfer with contiguous per-partition rows.
    xt_d = nc.declare_dram_parameter("xt", [128, KB, TC], bf16, isOutput=False)
    w13_d = nc.declare_dram_parameter(
        "w13p", [FB, 128, 2, H], bf16, isOutput=False
    )
    w2_d = nc.declare_dram_parameter(
        "w2p", [HCOLS, FB2, 128, 2, 512], bf16, isOutput=False
    )
    out_d = nc.declare_dram_parameter("out", [TC, H], f32, isOutput=True)

    with TileContext(nc) as tc:
        with (
            tc.tile_pool(name="xpool", bufs=1) as xpool,
            tc.tile_pool(name="wpool", bufs=4) as wpool,
            tc.tile_pool(name="w2pool", bufs=8) as w2pool,
            tc.tile_pool(name="sgpool", bufs=3) as sgpool,
            tc.tile_pool(name="upool", bufs=3) as upool,
            tc.tile_pool(name="fpool", bufs=FB) as fpool,
            tc.tile_pool(name="opool", bufs=HCOLS * TB) as opool,
            tc.tile_pool(name="psg", bufs=2, space="PSUM") as psg,
            tc.tile_pool(name="psu", bufs=2, space="PSUM") as psu,
            tc.tile_pool(name="psb", bufs=4, space="PSUM") as psb,
        ):
            xtile = xpool.tile([128, KB, TC], bf16)
            nc.sync.dma_start(out=xtile, in_=xt_d[:])

            fused = []
            for fb in range(FB):
                w13t = wpool.tile([128, 2, H], bf16, tag="w13t")
                nc.sync.dma_start(out=w13t, in_=w13_d[fb])

                gps = psg.tile([128, TC], f32, tag="gps")
                for kb in range(KB):
                    nc.tensor.matmul(
                        gps,
                        w13t[:, 0, kb * 128 : (kb + 1) * 128],
                        xtile[:, kb, :],
                        start=(kb == 0),
                        stop=(kb == KB - 1),
                    )
                ups = psu.tile([128, TC], f32, tag="ups")
                for kb in range(KB):
                    nc.tensor.matmul(
                        ups,
                        w13t[:, 1, kb * 128 : (kb + 1) * 128],
                        xtile[:, kb, :],
                        start=(kb == 0),
                        stop=(kb == KB - 1),
                    )

                # ACT evacuates both PSUM banks (Silu and Copy live in the
                # same ACT table, so alternating them reloads nothing); the
                # DVE multiply then depends on one engine only.
                sg = sgpool.tile([128, TC], f32, tag="sg")
                nc.scalar.activation(
                    sg, gps, mybir.ActivationFunctionType.Silu
                )
                usb = upool.tile([128, TC], f32, tag="usb")
                nc.scalar.copy(usb, ups)
                fut = fpool.tile(
                    [128, TC], bf16, tag="fused", name=f"fused{fb}"
                )
                nc.vector.tensor_tensor(
                    fut, sg, usb, mybir.AluOpType.mult
                )
                fused.append(fut)

            for hc in range(HCOLS):
                pss = []
                for tb in range(TB):
                    ps = psb.tile(
                        [128, 512], f32, tag="pss", name=f"pss{hc}_{tb}"
                    )
                    pss.append(ps)
                for j in range(FB2):
                    w2t = w2pool.tile([128, 2, 512], bf16, tag="w2t")
                    nc.sync.dma_start(out=w2t, in_=w2_d[hc, j])
                    for i in range(2):
                        fb = 2 * j + i
                        for tb in range(TB):
                            nc.tensor.matmul(
                                pss[tb],
                                fused[fb][:, tb * 128 : (tb + 1) * 128],
                                w2t[:, i, :],
                                start=(fb == 0),
                                stop=(fb == FB - 1),
                            )
                for tb in range(TB):
                    ot = opool.tile(
                        [128, 512], f32, tag="ot", name=f"ot{hc}_{tb}"
                    )
                    nc.vector.tensor_copy(ot, pss[tb])
                    nc.sync.dma_start(
                        out=out_d[
                            tb * 128 : (tb + 1) * 128,
                            hc * 512 : (hc + 1) * 512,
                        ],
                        in_=ot,
                    )
    nc.finalize()
    return nc


def _dequant(wq, s):
    wq = np.asarray(wq, dtype=np.float32)
    s = np.asarray(s, dtype=np.float32)
    n, k = wq.shape
    nb, kb = s.shape
    w = wq.reshape(nb, n // nb, kb, k // kb) * s[:, None, :, None]
    return w.reshape(n, k)


def _prep_inputs(hidden_states, w1_q, w1_s, w3_q, w3_s, w2_q, w2_s):
    import ml_dtypes

    bf = ml_dtypes.bfloat16

    w1 = _dequant(w1_q, w1_s).astype(bf)  # [F, H]
    w3 = _dequant(w3_q, w3_s).astype(bf)  # [F, H]
    w2 = _dequant(w2_q, w2_s).astype(bf)  # [H, F]

    # w1p[fb, p, kb*128+c] = w1[fb*128+c, kb*128+p]  (and same for w3);
    # interleaved per partition: w13p[fb, p, 0] = w1 row, [fb, p, 1] = w3.
    w1p = w1.reshape(FB, 128, KB, 128).transpose(0, 3, 2, 1).reshape(FB, 128, H)
    w3p = w3.reshape(FB, 128, KB, 128).transpose(0, 3, 2, 1).reshape(FB, 128, H)
    w13p = np.ascontiguousarray(np.stack([w1p, w3p], axis=2))  # [FB,128,2,H]

    # w2p[hc, j, p, i, c] = w2[hc*512+c, (2j+i)*128+p]
    w2p = np.ascontiguousarray(
        np.asarray(w2).reshape(HCOLS, 512, FB2, 2, 128).transpose(0, 2, 4, 3, 1)
    )

    x = np.asarray(hidden_states, dtype=np.float32).astype(bf)
    xts = []
    for c in range(NCORES):
        xc = x[c * TC : (c + 1) * TC, :]
        # xt[p, kb, t] = xc[t, kb*128+p] — partition-major, so the whole
        # 2MB x-transpose lands in one DMA with 16KB/partition contiguous.
        xts.append(
            np.ascontiguousarray(xc.reshape(TC, KB, 128).transpose(2, 1, 0))
        )

    return [
        {"xt": xts[c], "w13p": w13p, "w2p": w2p}
        for c in range(NCORES)
    ]


def _run(in_maps, **kwargs):
    from concourse.bass_utils import run_bass_kernel_spmd

    if "nc" not in _CACHE:
        _CACHE["nc"] = _build_program()
    res = run_bass_kernel_spmd(
        _CACHE["nc"], in_maps, list(range(NCORES)), **kwargs
    )
    out = np.concatenate(
        [res.results[c]["out"] for c in range(NCORES)], axis=0
    )
    return np.asarray(out, dtype=np.float32), res


def kernel(hidden_states, w1_q, w1_s, w3_q, w3_s, w2_q, w2_s):
    in_maps = _prep_inputs(
        hidden_states, w1_q, w1_s, w3_q, w3_s, w2_q, w2_s
    )
    out, _ = _run(in_maps)
    return out


# revision 16
# speedup vs baseline: 1.0371x; 1.0068x over previous
"""Fused FP8-block-quantized MLP (silu(x@w1.T) * (x@w3.T)) @ w2.T on 8 trn2 cores.

Sharding: data-parallel over tokens. Each core gets T/8 = 512 tokens and the
full (dequantized, bf16) weights; there are no collectives. Host-side prep
dequantizes the block-quantized weights, casts to bf16, and lays tensors out
partition-major so every device DMA is one large contiguous transfer.

Device kernel per core (all matmuls bf16, fp32 PSUM accumulation):
  phase A: for each 128-row block fb of F: g.T/u.T [128f, 512t] accumulated
           over 16 k-blocks of H; silu+copy on ACT, mul on DVE -> fusedT
           kept in SBUF.
  phase B: out [512t, 2048h] = fusedT.T @ w2.T, streaming w2 column blocks,
           accumulating over the 56 f-blocks in PSUM.
"""

import sys

import numpy as np

_REPO = "/opt/trn_rl_repo"
if _REPO not in sys.path:
    sys.path.insert(0, _REPO)

T, H, F = 4096, 2048, 7168
NCORES = 8
TC = T // NCORES      # 512 tokens per core
KB = H // 128         # 16 contraction blocks for matmul 1/3
FB = F // 128         # 56 f blocks
FB2 = FB // 2         # w2 blocks are streamed in pairs
HCOLS = H // 512      # 4 output column groups
TB = TC // 128        # 4 token blocks

_CACHE = {}


def _build_program():
    import concourse.mybir as mybir
    from concourse import bacc
    from concourse.tile import TileContext

    bf16 = mybir.dt.bfloat16
    f32 = mybir.dt.float32

    # Bacc (not bass.Bass): its finalize() runs generate_event_semaphores,
    # which splits multi-wait sync_info into EventSemaphore instructions —
    # TRN2 instructions physically carry at most one sem wait.
    nc = bacc.Bacc()
    # All inputs are laid out partition-major on the host so each DMA below
    # is a single large transfer with contiguous per-partition rows.
    xt_d = nc.declare_dram_parameter("xt", [128, KB, TC], bf16, isOutput=False)
    w13_d = nc.declare_dram_parameter(
        "w13p", [FB, 128, 2, H], bf16, isOutput=False
    )
    w2_d = nc.declare_dram_parameter(
        "w2p", [HCOLS, FB2, 128, 2, 512], bf16, isOutput=False
    )
    out_d = nc.declare_dram_parameter("out", [TC, H], f32, isOutput=True)

    with TileContext(nc) as tc:
        with (
            tc.tile_pool(name="xpool", bufs=1) as xpool,
            tc.tile_pool(name="wpool", bufs=4) as wpool,
            tc.tile_pool(name="w2pool", bufs=8) as w2pool,
            tc.tile_pool(name="sgpool", bufs=3) as sgpool,
            tc.tile_pool(name="upool", bufs=3) as upool,
            tc.tile_pool(name="fpool", bufs=FB) as fpool,
            tc.tile_pool(name="opool", bufs=HCOLS * TB) as opool,
            tc.tile_pool(name="psg", bufs=2, space="PSUM") as psg,
            tc.tile_pool(name="psu", bufs=2, space="PSUM") as psu,
            tc.tile_pool(name="psb", bufs=4, space="PSUM") as psb,
        ):
            xtile = xpool.tile([128, KB, TC], bf16)

            fused = []
            for fb in range(FB):
                w13t = wpool.tile([128, 2, H], bf16, tag="w13t")
                if fb == 0:
                    # Halve the startup loads so the first 8 k-blocks of
                    # matmuls start after ~1.5MB instead of the full 3MB,
                    # with arrivals still dense enough to keep HAM warm.
                    kh, hh = KB // 2, H // 2
                    nc.sync.dma_start(
                        out=xtile[:, :kh, :], in_=xt_d[:, :kh, :]
                    )
                    nc.sync.dma_start(
                        out=w13t[:, :, :hh], in_=w13_d[fb][:, :, :hh]
                    )
                    nc.sync.dma_start(
                        out=xtile[:, kh:, :], in_=xt_d[:, kh:, :]
                    )
                    nc.sync.dma_start(
                        out=w13t[:, :, hh:], in_=w13_d[fb][:, :, hh:]
                    )
                else:
                    nc.sync.dma_start(out=w13t, in_=w13_d[fb])

                gps = psg.tile([128, TC], f32, tag="gps")
                for kb in range(KB):
                    nc.tensor.matmul(
                        gps,
                        w13t[:, 0, kb * 128 : (kb + 1) * 128],
                        xtile[:, kb, :],
                        start=(kb == 0),
                        stop=(kb == KB - 1),
                    )
                ups = psu.tile([128, TC], f32, tag="ups")
                for kb in range(KB):
                    nc.tensor.matmul(
                        ups,
                        w13t[:, 1, kb * 128 : (kb + 1) * 128],
                        xtile[:, kb, :],
                        start=(kb == 0),
                        stop=(kb == KB - 1),
                    )

                # ACT evacuates both PSUM banks (Silu and Copy live in the
                # same ACT table, so alternating them reloads nothing); the
                # DVE multiply then depends on one engine only.
                sg = sgpool.tile([128, TC], f32, tag="sg")
                nc.scalar.activation(
                    sg, gps, mybir.ActivationFunctionType.Silu
                )
                usb = upool.tile([128, TC], f32, tag="usb")
                nc.scalar.copy(usb, ups)
                fut = fpool.tile(
                    [128, TC], bf16, tag="fused", name=f"fused{fb}"
                )
                nc.vector.tensor_tensor(
                    fut, sg, usb, mybir.AluOpType.mult
                )
                fused.append(fut)

            for hc in range(HCOLS):
                pss = []
                for tb in range(TB):
                    ps = psb.tile(
                        [128, 512], f32, tag="pss", name=f"pss{hc}_{tb}"
                    )
                    pss.append(ps)
                for j in range(FB2):
                    w2t = w2pool.tile([128, 2, 512], bf16, tag="w2t")
                    nc.sync.dma_start(out=w2t, in_=w2_d[hc, j])
                    for i in range(2):
                        fb = 2 * j + i
                        for tb in range(TB):
                            nc.tensor.matmul(
                                pss[tb],
                                fused[fb][:, tb * 128 : (tb + 1) * 128],
                                w2t[:, i, :],
                                start=(fb == 0),
                                stop=(fb == FB - 1),
                            )
                for tb in range(TB):
                    ot = opool.tile(
                        [128, 512], f32, tag="ot", name=f"ot{hc}_{tb}"
                    )
                    # Alternate DVE/ACT so the four evacuations drain in
                    # parallel; frees PSUM banks for the next hc sooner.
                    if tb % 2 == 0:
                        nc.vector.tensor_copy(ot, pss[tb])
                    else:
                        nc.scalar.copy(ot, pss[tb])
                    nc.sync.dma_start(
                        out=out_d[
                            tb * 128 : (tb + 1) * 128,
                            hc * 512 : (hc + 1) * 512,
                        ],
                        in_=ot,
                    )
    nc.finalize()
    return nc


def _dequant(wq, s):
    wq = np.asarray(wq, dtype=np.float32)
    s = np.asarray(s, dtype=np.float32)
    n, k = wq.shape
    nb, kb = s.shape
    w = wq.reshape(nb, n // nb, kb, k // kb) * s[:, None, :, None]
    return w.reshape(n, k)


def _prep_inputs(hidden_states, w1_q, w1_s, w3_q, w3_s, w2_q, w2_s):
    import ml_dtypes

    bf = ml_dtypes.bfloat16

    w1 = _dequant(w1_q, w1_s).astype(bf)  # [F, H]
    w3 = _dequant(w3_q, w3_s).astype(bf)  # [F, H]
    w2 = _dequant(w2_q, w2_s).astype(bf)  # [H, F]

    # w1p[fb, p, kb*128+c] = w1[fb*128+c, kb*128+p]  (and same for w3);
    # interleaved per partition: w13p[fb, p, 0] = w1 row, [fb, p, 1] = w3.
    w1p = w1.reshape(FB, 128, KB, 128).transpose(0, 3, 2, 1).reshape(FB, 128, H)
    w3p = w3.reshape(FB, 128, KB, 128).transpose(0, 3, 2, 1).reshape(FB, 128, H)
    w13p = np.ascontiguousarray(np.stack([w1p, w3p], axis=2))  # [FB,128,2,H]

    # w2p[hc, j, p, i, c] = w2[hc*512+c, (2j+i)*128+p]
    w2p = np.ascontiguousarray(
        np.asarray(w2).reshape(HCOLS, 512, FB2, 2, 128).transpose(0, 2, 4, 3, 1)
    )

    x = np.asarray(hidden_states, dtype=np.float32).astype(bf)
    xts = []
    for c in range(NCORES):
        xc = x[c * TC : (c + 1) * TC, :]
        # xt[p, kb, t] = xc[t, kb*128+p] — partition-major, so the whole
        # 2MB x-transpose lands in one DMA with 16KB/partition contiguous.
        xts.append(
            np.ascontiguousarray(xc.reshape(TC, KB, 128).transpose(2, 1, 0))
        )

    return [
        {"xt": xts[c], "w13p": w13p, "w2p": w2p}
        for c in range(NCORES)
    ]


def _run(in_maps, **kwargs):
    from concourse.bass_utils import run_bass_kernel_spmd

    if "nc" not in _CACHE:
        _CACHE["nc"] = _build_program()
    res = run_bass_kernel_spmd(
        _CACHE["nc"], in_maps, list(range(NCORES)), **kwargs
    )
    out = np.concatenate(
        [res.results[c]["out"] for c in range(NCORES)], axis=0
    )
    return np.asarray(out, dtype=np.float32), res


def kernel(hidden_states, w1_q, w1_s, w3_q, w3_s, w2_q, w2_s):
    in_maps = _prep_inputs(
        hidden_states, w1_q, w1_s, w3_q, w3_s, w2_q, w2_s
    )
    out, _ = _run(in_maps)
    return out
